# revision 23
# baseline (speedup 1.0000x reference)
"""BitNet linear layer (b1.58-style) on 8 Trainium2 NeuronCores.

Computes: scale = 1e-4 + mean(|W|); q = clip(round(W/scale), -1, 1);
          out = scale * (x @ q.T)
for x [4, 2048, 2048] f32 and W [8192, 2048] f32.

Sharding: tensor-parallel over out_features. Each core gets the full x
(replicated) and a 1024-row shard of W; host concatenates the 8 per-core
[8192, 1024] outputs along the feature axis.

On-device per core (v2 — PE runs only the main matmuls):
  - x is never cast on-device: the DMA reads the high 2 bytes of each f32
    (bf16 truncation, rel err ~3e-3 « 2e-2 gate) and the xbar DMA-transpose
    unit (InstDmaTransposeAnt) writes it straight into k-major xT tiles,
    [512 m-rows x 128 k] -> [128 k, 512 m] per instruction, 4 m-tiles per
    group instruction batch.  Zero PE / DVE / ACT work for the x pipeline.
  - W streams in 8 x 1 MiB chunks with pipelined |W| row reduces (DVE);
    the 4-byte AllReduce for the global absmean launches ~26us in.
  - thr = 0.5*scale doubles as the output scale: qT stores 2q (exact in
    bf16), the PSUM drain multiplies by thr = scale/2.
  - Quantize q2 = sign(W-thr) + sign(W+thr) on ACT for most n-tiles and
    2*[(W>thr) - (W<-thr)] on DVE for the rest, both engines racing in
    n-consumption order; qT transposes run on the (otherwise idle) PE.
  - Main loop: out[m, n] = sum_k xT[k, m] * qT[k, n] in PSUM over 16
    k-tiles per 512-wide n-half; a LAG-2 stagger between the two n-halves
    keeps the first m-tiles off the late qT half.  ACT drains PSUM fused
    with *thr; stores go out on the scalar queue.
"""

import sys

sys.path.insert(0, "/opt/trn_rl_repo")

import numpy as np
import ml_dtypes

import concourse.bass as bass
import concourse.tile as tile
from concourse import bacc, mybir
from concourse.bass_utils import run_bass_kernel_spmd
from concourse.masks import make_identity
from concourse import bass_isa

F32 = mybir.dt.float32
BF16 = mybir.dt.bfloat16

NCORES = 8
M = 8192          # tokens (4*2048)
K = 2048          # in_features
N_FULL = 8192     # out_features
NS = N_FULL // NCORES  # 1024 per-core shard
P = 128
KO = K // P       # 16 k-tiles
NO = NS // P      # 8 W-row tiles per shard
MT = M // P       # 64 m-tiles
W_ELEMS = float(N_FULL * K)  # 16777216, for the mean

GRP = 2           # m-tiles per x DMA-transpose group: p_dim = 256 rows
                  # = 16 xbar tiles per instruction, matching the +16
                  # semaphore increment tile assumes for a HWDGE DMA
NG = MT // GRP    # 32 groups
LAG = 2           # m-tiles between the nh0 and nh1 matmul passes
ACT_TILES = (0, 2, 4, 6, 7)   # quantized via two ACT sign passes
DVE_TILES = (1, 3, 5)         # quantized via DVE compares


def build_nc():
    nc = bacc.Bacc("TRN2", target_bir_lowering=False, debug=False,
                   num_devices=NCORES)
    # x is bound as the high half of each f32 word (round-toward-zero
    # bf16), gathered host-side during sharding.
    x_d = nc.dram_tensor("x", [M, K], BF16, kind="ExternalInput")
    w_d = nc.dram_tensor("w", [NS, K], F32, kind="ExternalInput")
    o_d = nc.dram_tensor("out", [M, NS], F32, kind="ExternalOutput")
    x_ap, w_ap, o_ap = x_d.ap(), w_d.ap(), o_d.ap()

    with tile.TileContext(nc) as tc:
        with (
            tc.tile_pool(name="const", bufs=1) as const,
            tc.tile_pool(name="scal", bufs=1) as scal,
            tc.tile_pool(name="wpool", bufs=8) as wpool,
            tc.tile_pool(name="qspool", bufs=3) as qspool,
            tc.tile_pool(name="qtpool", bufs=2) as qtpool,
            tc.tile_pool(name="qT_pool", bufs=1) as qT_pool,
            tc.tile_pool(name="xTpool", bufs=6) as xTpool,
            tc.tile_pool(name="opool", bufs=4) as opool,
            tc.tile_pool(name="psum_q", bufs=2, space="PSUM") as psum_q,
            tc.tile_pool(name="psum_o", bufs=4, space="PSUM") as psum_o,
            tc.tile_pool(name="dram", bufs=1, space="DRAM") as dram,
        ):
            ident = const.tile([P, P], BF16, name="ident")
            make_identity(nc, ident)

            # ---- W: 8 x 1MiB chunks, reduce |W| as each arrives --------
            wabs = scal.tile([P, NO], F32, name="wabs")
            w_tiles = {}
            for o in range(NO):
                wt = wpool.tile([P, K], F32, name=f"w_{o}", tag="w")
                nc.scalar.dma_start(wt[:], w_ap[o * P:(o + 1) * P, :])
                nc.vector.tensor_reduce(
                    wabs[:, o:o + 1], wt[:], mybir.AxisListType.X,
                    mybir.AluOpType.add, apply_absolute_value=True)
                w_tiles[o] = wt

            def emit_xgroup(g):
                # One xbar-transpose instruction per group:
                # out[p, kt, m] = x[m0+m, kt*128+p].  The destination is the
                # whole tile (fully contiguous per partition) — sliced
                # destinations are known to produce wrong output on HW.
                # All transposes stay on one queue (two xbar transposes
                # running concurrently on different queues corrupt data),
                # and none is emitted before the W stream + collective
                # launch: tile serializes DMA-transpose against every other
                # DMA, so an early transpose would chain-block the W loads.
                xg = xTpool.tile([P, KO, GRP * P], BF16, name=f"xT_{g}",
                                 tag="xT")
                m0 = g * GRP * P
                nc.sync.dma_start_transpose(xg[:], x_ap[m0:m0 + GRP * P, :])
                return xg

            # ---- global scale -----------------------------------------
            wsum = scal.tile([P, 1], F32, name="wsum")
            nc.vector.tensor_reduce(
                wsum[:], wabs[:], mybir.AxisListType.X, mybir.AluOpType.add)
            tot128 = scal.tile([P, 1], F32, name="tot128")
            nc.gpsimd.partition_all_reduce(
                tot128[:], wsum[:], P, bass_isa.ReduceOp.add)

            cc_in = dram.tile([1, 1], F32, name="cc_in")
            cc_out = dram.tile([1, 1], F32, name="cc_out", addr_space="Shared")
            nc.gpsimd.dma_start(cc_in[:], tot128[0:1, :])
            nc.gpsimd.collective_compute(
                "AllReduce", mybir.AluOpType.add,
                replica_groups=[list(range(NCORES))],
                ins=[cc_in[:].opt()], outs=[cc_out[:].opt()])

            # x prefetch overlaps the collective wait; emitted after the
            # collective launch so the DMA serialization chain (every DMA
            # serializes pairwise with a DMA-transpose) runs
            # W -> cc_in -> doorbell -> transposes -> result fetch.
            xgroups = {g: emit_xgroup(g) for g in range(4)}

            tot_sb = scal.tile([1, 1], F32, name="tot_sb")
            nc.gpsimd.dma_start(tot_sb[:], cc_out[:])
            bcast = scal.tile([P, 1], F32, name="bcast")
            nc.gpsimd.partition_broadcast(bcast[:], tot_sb[:])

            # thr = 0.5*scale = 0.5e-4 + tot/(2*W_ELEMS); also the output
            # scale because qT holds 2q.
            thr_pos = scal.tile([P, 1], F32, name="thr_pos")
            nc.vector.tensor_scalar(
                thr_pos[:], bcast[:], 0.5 / W_ELEMS, 0.5e-4,
                mybir.AluOpType.mult, mybir.AluOpType.add)
            thr_neg = scal.tile([P, 1], F32, name="thr_neg")
            nc.vector.tensor_scalar(
                thr_neg[:], thr_pos[:], -1.0, None, mybir.AluOpType.mult)

            # ---- quantize (2q) + PE transpose -> qT [P, KO, NS] -------
            qT = qT_pool.tile([P, KO, NS], BF16, name="qT")

            def emit_quant(o):
                wt = w_tiles[o]
                q2 = qtpool.tile([P, K], BF16, name=f"q2_{o}", tag="q2")
                if o in ACT_TILES:
                    s1 = qspool.tile([P, K], BF16, name=f"s1_{o}", tag="qs")
                    nc.scalar.activation(
                        s1[:], wt[:], mybir.ActivationFunctionType.Sign,
                        bias=thr_neg[:])
                    nc.scalar.activation(
                        q2[:], wt[:], mybir.ActivationFunctionType.Sign,
                        bias=thr_pos[:])
                    nc.vector.tensor_tensor(
                        q2[:], q2[:], s1[:], mybir.AluOpType.add)
                else:
                    gt = qspool.tile([P, K], BF16, name=f"gt_{o}", tag="qs")
                    nc.vector.tensor_scalar(
                        gt[:], wt[:], thr_pos[:], 2.0,
                        mybir.AluOpType.is_gt, mybir.AluOpType.mult)
                    nc.vector.tensor_scalar(
                        q2[:], wt[:], thr_neg[:], -2.0,
                        mybir.AluOpType.is_lt, mybir.AluOpType.mult)
                    nc.vector.tensor_tensor(
                        q2[:], q2[:], gt[:], mybir.AluOpType.add)
                for g in range(4):
                    pt = psum_q.tile([P, 4 * P], F32, name=f"ptq_{o}_{g}",
                                     tag="ptq")
                    for j in range(4):
                        ko = g * 4 + j
                        nc.tensor.matmul(
                            pt[:, j * P:(j + 1) * P],
                            lhsT=q2[:, ko * P:(ko + 1) * P],
                            rhs=ident[:], start=True, stop=True)
                    dst = qT[:, g * 4:(g + 1) * 4, o * P:(o + 1) * P]
                    src = pt[:].rearrange("p (a b) -> p a b", a=4)
                    if g % 2:
                        nc.scalar.copy(dst, src)
                    else:
                        nc.vector.tensor_copy(dst, src)

            # n-consumption order: nh0 tiles 0-3 first.
            for o in (0, 1, 2, 3, 4, 5, 6, 7):
                emit_quant(o)

            # ---- main loop --------------------------------------------
            def mm_half(mt, nh):
                g, mloc = divmod(mt, GRP)
                xg = xgroups[g]
                po = psum_o.tile([P, 512], F32, name=f"po_{mt}_{nh}",
                                 tag="po")
                for ko in range(KO):
                    nc.tensor.matmul(
                        po[:], lhsT=xg[:, ko, mloc * P:(mloc + 1) * P],
                        rhs=qT[:, ko, nh * 512:(nh + 1) * 512],
                        start=(ko == 0), stop=(ko == KO - 1))
                ot = out_tiles[mt]
                nc.scalar.activation(
                    ot[:, nh * 512:(nh + 1) * 512], po[:],
                    mybir.ActivationFunctionType.Copy, scale=thr_pos[:])
                if nh == 1:
                    nc.scalar.dma_start(
                        o_ap[mt * P:(mt + 1) * P, :], ot[:])

            out_tiles = {}
            for mt in range(MT):
                if mt % GRP == 0:
                    gpre = min(mt // GRP + 3, NG - 1)
                    for gg in range(1, gpre + 1):
                        if gg not in xgroups:
                            xgroups[gg] = emit_xgroup(gg)
                out_tiles[mt] = opool.tile([P, NS], F32, name=f"o_{mt}",
                                           tag="o")
                mm_half(mt, 0)
                if mt >= LAG:
                    mm_half(mt - LAG, 1)
            for mt in range(MT - LAG, MT):
                mm_half(mt, 1)

    nc.compile()
    return nc


_NC_CACHE = None


def get_nc():
    global _NC_CACHE
    if _NC_CACHE is None:
        _NC_CACHE = build_nc()
    return _NC_CACHE


def make_in_maps(x, weight):
    x2 = np.ascontiguousarray(np.asarray(x, dtype=np.float32).reshape(M, K))
    # byte gather: high half of each little-endian f32 word == bf16
    # truncation of x (no arithmetic happens on the host)
    xv = np.ascontiguousarray(x2.view(ml_dtypes.bfloat16)[:, 1::2])
    w = np.asarray(weight, dtype=np.float32)
    return [
        {"x": xv, "w": np.ascontiguousarray(w[c * NS:(c + 1) * NS])}
        for c in range(NCORES)
    ]


def kernel(x, weight):
    nc = get_nc()
    in_maps = make_in_maps(x, weight)
    try:
        res = run_bass_kernel_spmd(nc, in_maps, list(range(NCORES)))
    except Exception:
        # transient device errors have been observed on first touch; retry once
        res = run_bass_kernel_spmd(nc, in_maps, list(range(NCORES)))
    out = np.concatenate(
        [res.results[c]["out"] for c in range(NCORES)], axis=1)
    return np.ascontiguousarray(out.reshape(4, 2048, N_FULL), dtype=np.float32)


# revision 24
# speedup vs baseline: 1.0672x; 1.0672x over previous
"""BitNet linear layer (b1.58-style) on 8 Trainium2 NeuronCores.

Computes: scale = 1e-4 + mean(|W|); q = clip(round(W/scale), -1, 1);
          out = scale * (x @ q.T)
for x [4, 2048, 2048] f32 and W [8192, 2048] f32.

Sharding: tensor-parallel over out_features. Each core gets the full x
(replicated) and a 1024-row shard of W; host concatenates the 8 per-core
[8192, 1024] outputs along the feature axis.

On-device per core (v2 — PE runs only the main matmuls):
  - x is never cast on-device: the DMA reads the high 2 bytes of each f32
    (bf16 truncation, rel err ~3e-3 « 2e-2 gate) and the xbar DMA-transpose
    unit (InstDmaTransposeAnt) writes it straight into k-major xT tiles,
    [512 m-rows x 128 k] -> [128 k, 512 m] per instruction, 4 m-tiles per
    group instruction batch.  Zero PE / DVE / ACT work for the x pipeline.
  - W streams in 8 x 1 MiB chunks with pipelined |W| row reduces (DVE);
    the 4-byte AllReduce for the global absmean launches ~26us in.
  - thr = 0.5*scale doubles as the output scale: qT stores 2q (exact in
    bf16), the PSUM drain multiplies by thr = scale/2.
  - Quantize q2 = sign(W-thr) + sign(W+thr) on ACT for most n-tiles and
    2*[(W>thr) - (W<-thr)] on DVE for the rest, both engines racing in
    n-consumption order; qT transposes run on the (otherwise idle) PE.
  - Main loop: out[m, n] = sum_k xT[k, m] * qT[k, n] in PSUM over 16
    k-tiles per 512-wide n-half; a LAG-2 stagger between the two n-halves
    keeps the first m-tiles off the late qT half.  ACT drains PSUM fused
    with *thr; stores go out on the scalar queue.
"""

import sys

sys.path.insert(0, "/opt/trn_rl_repo")

import numpy as np
import ml_dtypes

import concourse.bass as bass
import concourse.tile as tile
from concourse import bacc, mybir
from concourse.bass_utils import run_bass_kernel_spmd
from concourse.masks import make_identity
from concourse import bass_isa

F32 = mybir.dt.float32
BF16 = mybir.dt.bfloat16

NCORES = 8
M = 8192          # tokens (4*2048)
K = 2048          # in_features
N_FULL = 8192     # out_features
NS = N_FULL // NCORES  # 1024 per-core shard
P = 128
KO = K // P       # 16 k-tiles
NO = NS // P      # 8 W-row tiles per shard
MT = M // P       # 64 m-tiles
W_ELEMS = float(N_FULL * K)  # 16777216, for the mean

GRP = 2           # m-tiles per x DMA-transpose group: p_dim = 256 rows
                  # = 16 xbar tiles per instruction, matching the +16
                  # semaphore increment tile assumes for a HWDGE DMA
NG = MT // GRP    # 32 groups
LAG = 2           # m-tiles between the nh0 and nh1 matmul passes
ACT_TILES = (0, 2, 4, 6, 7)   # quantized via two ACT sign passes
DVE_TILES = (1, 3, 5)         # quantized via DVE compares


def build_nc():
    nc = bacc.Bacc("TRN2", target_bir_lowering=False, debug=False,
                   num_devices=NCORES)
    # x is bound as the high half of each f32 word (round-toward-zero
    # bf16), gathered host-side during sharding.
    x_d = nc.dram_tensor("x", [M, K], BF16, kind="ExternalInput")
    w_d = nc.dram_tensor("w", [NS, K], F32, kind="ExternalInput")
    o_d = nc.dram_tensor("out", [M, NS], F32, kind="ExternalOutput")
    x_ap, w_ap, o_ap = x_d.ap(), w_d.ap(), o_d.ap()

    with tile.TileContext(nc) as tc:
        with (
            tc.tile_pool(name="const", bufs=1) as const,
            tc.tile_pool(name="scal", bufs=1) as scal,
            tc.tile_pool(name="wpool", bufs=8) as wpool,
            tc.tile_pool(name="qspool", bufs=3) as qspool,
            tc.tile_pool(name="qtpool", bufs=2) as qtpool,
            tc.tile_pool(name="qT_pool", bufs=1) as qT_pool,
            tc.tile_pool(name="xTpool", bufs=6) as xTpool,
            tc.tile_pool(name="opool", bufs=4) as opool,
            tc.tile_pool(name="psum_q", bufs=2, space="PSUM") as psum_q,
            tc.tile_pool(name="psum_o", bufs=4, space="PSUM") as psum_o,
            tc.tile_pool(name="dram", bufs=1, space="DRAM") as dram,
        ):
            ident = const.tile([P, P], BF16, name="ident")
            make_identity(nc, ident)

            # ---- W: 8 x 1MiB chunks, reduce |W| as each arrives --------
            wabs = scal.tile([P, NO], F32, name="wabs")
            w_tiles = {}
            for o in range(NO):
                wt = wpool.tile([P, K], F32, name=f"w_{o}", tag="w")
                nc.scalar.dma_start(wt[:], w_ap[o * P:(o + 1) * P, :])
                nc.vector.tensor_reduce(
                    wabs[:, o:o + 1], wt[:], mybir.AxisListType.X,
                    mybir.AluOpType.add, apply_absolute_value=True)
                w_tiles[o] = wt

            def emit_xgroup(g):
                # One xbar-transpose instruction per group:
                # out[p, kt, m] = x[m0+m, kt*128+p].  The destination is the
                # whole tile (fully contiguous per partition) — sliced
                # destinations are known to produce wrong output on HW.
                # All transposes stay on one queue (two xbar transposes
                # running concurrently on different queues corrupt data).
                # Tile serializes every DMA pairwise with a DMA-transpose
                # into one chain whose order the scheduler picks freely, so
                # each transpose gets a WAW marker dep on tot128 — without
                # it the scheduler interleaves transposes into the W stream
                # and delays the collective by ~60us.
                xg = xTpool.tile([P, KO, GRP * P], BF16, name=f"xT_{g}",
                                 tag="xT")
                nc.vector.tensor_scalar(
                    xg[0:1, 0, 0:1], tot128[0:1, :], 0.0, None,
                    mybir.AluOpType.mult)
                m0 = g * GRP * P
                nc.sync.dma_start_transpose(xg[:], x_ap[m0:m0 + GRP * P, :])
                return xg

            # ---- global scale -----------------------------------------
            wsum = scal.tile([P, 1], F32, name="wsum")
            nc.vector.tensor_reduce(
                wsum[:], wabs[:], mybir.AxisListType.X, mybir.AluOpType.add)
            tot128 = scal.tile([P, 1], F32, name="tot128")
            nc.gpsimd.partition_all_reduce(
                tot128[:], wsum[:], P, bass_isa.ReduceOp.add)

            cc_in = dram.tile([1, 1], F32, name="cc_in")
            cc_out = dram.tile([1, 1], F32, name="cc_out", addr_space="Shared")
            nc.gpsimd.dma_start(cc_in[:], tot128[0:1, :])
            nc.gpsimd.collective_compute(
                "AllReduce", mybir.AluOpType.add,
                replica_groups=[list(range(NCORES))],
                ins=[cc_in[:].opt()], outs=[cc_out[:].opt()])

            # x prefetch overlaps the collective wait; emitted after the
            # collective launch so the DMA serialization chain (every DMA
            # serializes pairwise with a DMA-transpose) runs
            # W -> cc_in -> doorbell -> transposes -> result fetch.
            xgroups = {g: emit_xgroup(g) for g in range(4)}

            tot_sb = scal.tile([1, 1], F32, name="tot_sb")
            nc.gpsimd.dma_start(tot_sb[:], cc_out[:])
            bcast = scal.tile([P, 1], F32, name="bcast")
            nc.gpsimd.partition_broadcast(bcast[:], tot_sb[:])

            # thr = 0.5*scale = 0.5e-4 + tot/(2*W_ELEMS); also the output
            # scale because qT holds 2q.
            thr_pos = scal.tile([P, 1], F32, name="thr_pos")
            nc.vector.tensor_scalar(
                thr_pos[:], bcast[:], 0.5 / W_ELEMS, 0.5e-4,
                mybir.AluOpType.mult, mybir.AluOpType.add)
            thr_neg = scal.tile([P, 1], F32, name="thr_neg")
            nc.vector.tensor_scalar(
                thr_neg[:], thr_pos[:], -1.0, None, mybir.AluOpType.mult)

            # ---- quantize (2q) + PE transpose -> qT [P, KO, NS] -------
            qT = qT_pool.tile([P, KO, NS], BF16, name="qT")

            def emit_quant(o):
                wt = w_tiles[o]
                q2 = qtpool.tile([P, K], BF16, name=f"q2_{o}", tag="q2")
                if o in ACT_TILES:
                    s1 = qspool.tile([P, K], BF16, name=f"s1_{o}", tag="qs")
                    nc.scalar.activation(
                        s1[:], wt[:], mybir.ActivationFunctionType.Sign,
                        bias=thr_neg[:])
                    nc.scalar.activation(
                        q2[:], wt[:], mybir.ActivationFunctionType.Sign,
                        bias=thr_pos[:])
                    nc.vector.tensor_tensor(
                        q2[:], q2[:], s1[:], mybir.AluOpType.add)
                else:
                    gt = qspool.tile([P, K], BF16, name=f"gt_{o}", tag="qs")
                    nc.vector.tensor_scalar(
                        gt[:], wt[:], thr_pos[:], 2.0,
                        mybir.AluOpType.is_gt, mybir.AluOpType.mult)
                    nc.vector.tensor_scalar(
                        q2[:], wt[:], thr_neg[:], -2.0,
                        mybir.AluOpType.is_lt, mybir.AluOpType.mult)
                    nc.vector.tensor_tensor(
                        q2[:], q2[:], gt[:], mybir.AluOpType.add)
                for g in range(4):
                    pt = psum_q.tile([P, 4 * P], F32, name=f"ptq_{o}_{g}",
                                     tag="ptq")
                    for j in range(4):
                        ko = g * 4 + j
                        nc.tensor.matmul(
                            pt[:, j * P:(j + 1) * P],
                            lhsT=q2[:, ko * P:(ko + 1) * P],
                            rhs=ident[:], start=True, stop=True)
                    dst = qT[:, g * 4:(g + 1) * 4, o * P:(o + 1) * P]
                    src = pt[:].rearrange("p (a b) -> p a b", a=4)
                    if g % 2:
                        nc.scalar.copy(dst, src)
                    else:
                        nc.vector.tensor_copy(dst, src)

            # n-consumption order: nh0 tiles 0-3 first.
            for o in (0, 1, 2, 3, 4, 5, 6, 7):
                emit_quant(o)

            # ---- main loop --------------------------------------------
            def mm_half(mt, nh):
                g, mloc = divmod(mt, GRP)
                xg = xgroups[g]
                po = psum_o.tile([P, 512], F32, name=f"po_{mt}_{nh}",
                                 tag="po")
                for ko in range(KO):
                    nc.tensor.matmul(
                        po[:], lhsT=xg[:, ko, mloc * P:(mloc + 1) * P],
                        rhs=qT[:, ko, nh * 512:(nh + 1) * 512],
                        start=(ko == 0), stop=(ko == KO - 1))
                ot = out_tiles[mt]
                nc.scalar.activation(
                    ot[:, nh * 512:(nh + 1) * 512], po[:],
                    mybir.ActivationFunctionType.Copy, scale=thr_pos[:])
                if nh == 1:
                    nc.scalar.dma_start(
                        o_ap[mt * P:(mt + 1) * P, :], ot[:])

            out_tiles = {}
            for mt in range(MT):
                if mt % GRP == 0:
                    gpre = min(mt // GRP + 3, NG - 1)
                    for gg in range(1, gpre + 1):
                        if gg not in xgroups:
                            xgroups[gg] = emit_xgroup(gg)
                out_tiles[mt] = opool.tile([P, NS], F32, name=f"o_{mt}",
                                           tag="o")
                mm_half(mt, 0)
                if mt >= LAG:
                    mm_half(mt - LAG, 1)
            for mt in range(MT - LAG, MT):
                mm_half(mt, 1)

    nc.compile()
    return nc


_NC_CACHE = None


def get_nc():
    global _NC_CACHE
    if _NC_CACHE is None:
        _NC_CACHE = build_nc()
    return _NC_CACHE


def make_in_maps(x, weight):
    x2 = np.ascontiguousarray(np.asarray(x, dtype=np.float32).reshape(M, K))
    # byte gather: high half of each little-endian f32 word == bf16
    # truncation of x (no arithmetic happens on the host)
    xv = np.ascontiguousarray(x2.view(ml_dtypes.bfloat16)[:, 1::2])
    w = np.asarray(weight, dtype=np.float32)
    return [
        {"x": xv, "w": np.ascontiguousarray(w[c * NS:(c + 1) * NS])}
        for c in range(NCORES)
    ]


def kernel(x, weight):
    nc = get_nc()
    in_maps = make_in_maps(x, weight)
    try:
        res = run_bass_kernel_spmd(nc, in_maps, list(range(NCORES)))
    except Exception:
        # transient device errors have been observed on first touch; retry once
        res = run_bass_kernel_spmd(nc, in_maps, list(range(NCORES)))
    out = np.concatenate(
        [res.results[c]["out"] for c in range(NCORES)], axis=1)
    return np.ascontiguousarray(out.reshape(4, 2048, N_FULL), dtype=np.float32)
